# revision 1
# baseline (speedup 1.0000x reference)
"""RBF Gram matrix kernel for Trainium2, 8-core SPMD.

K[i, j] = exp(-gamma * ||x_i - s_j||^2),  x [8192, 256] f32, support [8192, 256] f32.

Strategy:
  - Shard rows of x across 8 cores (1024 rows/core); replicate support.
  - exponent = 2*gamma * (x.s - 0.5*||x||^2 - 0.5*||s||^2).  The norm terms are
    folded into the GEMM as 4 extra contraction rows (hi/lo fp16 splits of
    -0.5*||.||^2 paired with ones), so the epilogue is a single ScalarE
    activation: out = Exp(2*gamma * psum).
  - Operands pre-transposed and cast to fp16 on host (K on partitions);
    PSUM accumulates fp32; output written f32.
"""

import numpy as np

try:
    import concourse.bass as bass  # noqa: F401
except ImportError:
    import sys

    sys.path.insert(0, "/opt/trn_rl_repo")

N, M, D = 8192, 8192, 256
GAMMA = 1.0 / D
NCORES = 8
STRIP = N // NCORES  # 1024 rows of x per core
P = 128
NTILE = 512  # matmul free-dim slice (one fp32 PSUM bank)
NGROUP = 2048  # activation/store group: 4 PSUM banks per ACTIVATE + one 1MB store

_CACHE = {}


def _build():
    import concourse.tile as tile
    from concourse import bacc, mybir

    f16 = mybir.dt.float16
    f32 = mybir.dt.float32

    nc = bacc.Bacc("TRN2", target_bir_lowering=False, debug=False, num_devices=NCORES)

    xa = nc.dram_tensor("xa", [2, P, STRIP], f16, kind="ExternalInput")
    xg = nc.dram_tensor("xg", [4, STRIP], f16, kind="ExternalInput")
    sa = nc.dram_tensor("sa", [2, P, M], f16, kind="ExternalInput")
    sg = nc.dram_tensor("sg", [4, M], f16, kind="ExternalInput")
    out = nc.dram_tensor("out", [STRIP, M], f32, kind="ExternalOutput")

    n_mt = STRIP // P  # 8 m-tiles
    n_nt = M // NTILE  # 16 n-tiles

    with tile.TileContext(nc) as tc:
        with (
            tc.tile_pool(name="const", bufs=1) as const,
            tc.tile_pool(name="psum", bufs=2, space="PSUM") as psum_pool,
            tc.tile_pool(name="obuf", bufs=6) as obuf,
        ):
            xa_t = []
            for c in range(2):
                t = const.tile([P, STRIP], f16, tag=f"xa{c}")
                nc.sync.dma_start(out=t[:], in_=xa[c])
                xa_t.append(t)
            xg_t = const.tile([4, STRIP], f16, tag="xg")
            nc.sync.dma_start(out=xg_t[:], in_=xg[:])
            sg_t = const.tile([4, M], f16, tag="sg")
            nc.sync.dma_start(out=sg_t[:], in_=sg[:])
            sa_t = {}
            for n in range(n_nt):
                for c in range(2):
                    t = const.tile([P, NTILE], f16, tag=f"sa{c}_{n}")
                    nc.sync.dma_start(
                        out=t[:], in_=sa[c, :, n * NTILE : (n + 1) * NTILE]
                    )
                    sa_t[c, n] = t

            GW = NGROUP // NTILE  # matmul slices per activation/store group
            n_grp = M // NGROUP
            for m in range(n_mt):
                ms = slice(m * P, (m + 1) * P)
                for g in range(n_grp):
                    ps = psum_pool.tile([P, NGROUP], f32)
                    # chunk-outer: one LDWEIGHTS serves GW matmuls
                    for c in range(3):
                        lhsT = xg_t[:, ms] if c == 2 else xa_t[c][:, ms]
                        for k in range(GW):
                            n = g * GW + k
                            ks = slice(k * NTILE, (k + 1) * NTILE)
                            if c == 2:
                                rhs = sg_t[:, n * NTILE : (n + 1) * NTILE]
                            else:
                                rhs = sa_t[c, n][:]
                            nc.tensor.matmul(
                                ps[:, ks],
                                lhsT,
                                rhs,
                                start=(c == 0),
                                stop=(c == 2),
                            )
                    ot = obuf.tile([P, NGROUP], f32)
                    nc.scalar.activation(
                        ot[:],
                        ps[:],
                        mybir.ActivationFunctionType.Exp,
                        bias=0.0,
                        scale=2.0 * GAMMA,
                    )
                    gs = slice(g * NGROUP, (g + 1) * NGROUP)
                    eng = nc.gpsimd if (m * n_grp + g) % 2 == 0 else nc.sync
                    eng.dma_start(out=out[ms, gs], in_=ot[:])
    nc.compile()
    return nc


def _hi_lo(v):
    """Split f32 vector into fp16 hi + lo so hi+lo ~= v to ~2^-21 relative."""
    hi = v.astype(np.float16)
    lo = (v - hi.astype(np.float32)).astype(np.float16)
    return hi, lo


def kernel(x, support):
    from concourse.bass_utils import run_bass_kernel_spmd

    if "nc" not in _CACHE:
        _CACHE["nc"] = _build()
    nc = _CACHE["nc"]

    x = np.asarray(x, dtype=np.float32)
    support = np.asarray(support, dtype=np.float32)

    x_sq = np.einsum("nd,nd->n", x, x)
    s_sq = np.einsum("md,md->m", support, support)
    ones_n = np.ones(N, np.float16)
    ones_m = np.ones(M, np.float16)

    # [256, 8192] fp16, K on rows; split into 2 chunks of 128
    xT = np.ascontiguousarray(x.T.astype(np.float16)).reshape(2, P, N)
    sT = np.ascontiguousarray(support.T.astype(np.float16)).reshape(2, P, M)

    xhi, xlo = _hi_lo(-0.5 * x_sq)
    shi, slo = _hi_lo(-0.5 * s_sq)
    # aug row pairing: (xhi,1) (xlo,1) (1,shi) (1,slo)
    xg_full = np.ascontiguousarray(np.stack([xhi, xlo, ones_n, ones_n]))
    sg = np.ascontiguousarray(np.stack([ones_m, ones_m, shi, slo]))

    in_maps = []
    for c in range(NCORES):
        cs = slice(c * STRIP, (c + 1) * STRIP)
        in_maps.append(
            {
                "xa": np.ascontiguousarray(xT[:, :, cs]),
                "xg": np.ascontiguousarray(xg_full[:, cs]),
                "sa": sT,
                "sg": sg,
            }
        )

    res = run_bass_kernel_spmd(nc, in_maps, list(range(NCORES)))
    return np.concatenate([res.results[c]["out"] for c in range(NCORES)], axis=0)



# revision 3
# speedup vs baseline: 1.4107x; 1.4107x over previous
"""RBF Gram matrix kernel for Trainium2, 8-core SPMD.

K[i, j] = exp(-gamma * ||x_i - s_j||^2),  x [8192, 256] f32, support [8192, 256] f32.

Strategy (v2):
  - 4x2 shard grid: x rows split into 4 strips of 2048, support cols into 2
    halves of 4096. Core (r, h) computes the [2048, 4096] block.
  - exp(-g*||x-s||^2) = exp(2g*x.s - g*||x||^2) * exp(-g*||s||^2).
    The GEMM computes x.s only (2 chunks of K=128, fp16); the row term rides
    the ScalarE activation as a per-partition bias; the column factor is a
    single fp16 VectorE tensor_tensor multiply against a precomputed
    broadcast tile. No augmented-row matmuls.
  - Output written fp16 (halves store traffic vs f32); host upcasts.
"""

import numpy as np

try:
    import concourse.bass as bass  # noqa: F401
except ImportError:
    import sys

    sys.path.insert(0, "/opt/trn_rl_repo")

N, M, D = 8192, 8192, 256
GAMMA = 1.0 / D
NCORES = 8
RSH, CSH = 4, 2  # row shards x col shards
SR = N // RSH  # 2048 x-rows per core
SC = M // CSH  # 4096 support-cols per core
P = 128
NTILE = 512  # matmul free-dim slice
NGROUP = 2048  # PSUM group: 4 banks, one ACTIVATE + one DVE mult per group

_CACHE = {}


def _build():
    import concourse.tile as tile
    from concourse import bacc, mybir

    f16 = mybir.dt.float16
    f32 = mybir.dt.float32

    nc = bacc.Bacc("TRN2", target_bir_lowering=False, debug=False, num_devices=NCORES)

    xa = nc.dram_tensor("xa", [2, P, SR], f16, kind="ExternalInput")
    sa = nc.dram_tensor("sa", [2, P, SC], f16, kind="ExternalInput")
    cb = nc.dram_tensor("cb", [P, SC], f16, kind="ExternalInput")
    xb = nc.dram_tensor("xb", [P, SR // P], f32, kind="ExternalInput")
    out = nc.dram_tensor("out", [SR // P, P, SC], f16, kind="ExternalOutput")

    n_mt = SR // P  # 16 m-tiles
    n_grp = SC // NGROUP  # 2 col groups per m-tile
    gw = NGROUP // NTILE  # 4 matmul slices per group

    with tile.TileContext(nc) as tc:
        with (
            tc.tile_pool(name="const", bufs=1) as const,
            tc.tile_pool(name="psum", bufs=2, space="PSUM") as psum_pool,
            tc.tile_pool(name="ebuf", bufs=4) as ebuf,
            tc.tile_pool(name="obuf", bufs=3) as obuf,
        ):
            # Interleave loads so the first matmul's operands arrive first.
            xa_t, sa_t = [], []
            for c in range(2):
                t = const.tile([P, SR], f16, tag=f"xa{c}")
                nc.sync.dma_start(out=t[:], in_=xa[c])
                xa_t.append(t)
                t2 = const.tile([P, SC], f16, tag=f"sa{c}")
                nc.sync.dma_start(out=t2[:], in_=sa[c])
                sa_t.append(t2)
            xb_t = const.tile([P, SR // P], f32, tag="xb")
            nc.sync.dma_start(out=xb_t[:], in_=xb[:])
            cb_t = const.tile([P, SC], f16, tag="cb")
            nc.sync.dma_start(out=cb_t[:], in_=cb[:])

            for m in range(n_mt):
                ms = slice(m * P, (m + 1) * P)
                ot = obuf.tile([P, SC], f16)
                for g in range(n_grp):
                    ps = psum_pool.tile([P, NGROUP], f32)
                    # chunk-outer: one LDWEIGHTS serves gw matmuls
                    for c in range(2):
                        lhsT = xa_t[c][:, ms]
                        for k in range(gw):
                            ns = slice(
                                g * NGROUP + k * NTILE, g * NGROUP + (k + 1) * NTILE
                            )
                            nc.tensor.matmul(
                                ps[:, k * NTILE : (k + 1) * NTILE],
                                lhsT,
                                sa_t[c][:, ns],
                                start=(c == 0),
                                stop=(c == 1),
                            )
                    et = ebuf.tile([P, NGROUP], f16)
                    nc.scalar.activation(
                        et[:],
                        ps[:],
                        mybir.ActivationFunctionType.Exp,
                        bias=xb_t[:, m : m + 1],
                        scale=2.0 * GAMMA,
                    )
                    gs = slice(g * NGROUP, (g + 1) * NGROUP)
                    nc.vector.tensor_tensor(
                        out=ot[:, gs],
                        in0=et[:],
                        in1=cb_t[:, gs],
                        op=mybir.AluOpType.mult,
                    )
                nc.sync.dma_start(out=out[m], in_=ot[:])
    nc.compile()
    return nc


def kernel(x, support):
    from concourse.bass_utils import run_bass_kernel_spmd

    if "nc" not in _CACHE:
        _CACHE["nc"] = _build()
    nc = _CACHE["nc"]

    x = np.asarray(x, dtype=np.float32)
    support = np.asarray(support, dtype=np.float32)

    x_sq = np.einsum("nd,nd->n", x, x)
    s_sq = np.einsum("md,md->m", support, support)

    # [256, N] fp16, contraction on rows; split into 2 chunks of 128
    xT = np.ascontiguousarray(x.T.astype(np.float16)).reshape(2, P, N)
    sT = np.ascontiguousarray(support.T.astype(np.float16)).reshape(2, P, M)

    col_factor = np.exp(-GAMMA * s_sq).astype(np.float16)  # [M]
    row_bias = (-GAMMA * x_sq).astype(np.float32)  # [N]

    xa_r = [np.ascontiguousarray(xT[:, :, r * SR : (r + 1) * SR]) for r in range(RSH)]
    xb_r = [
        np.ascontiguousarray(row_bias[r * SR : (r + 1) * SR].reshape(SR // P, P).T)
        for r in range(RSH)
    ]
    sa_h = [np.ascontiguousarray(sT[:, :, h * SC : (h + 1) * SC]) for h in range(CSH)]
    cb_h = [
        np.ascontiguousarray(
            np.broadcast_to(col_factor[h * SC : (h + 1) * SC], (P, SC))
        )
        for h in range(CSH)
    ]

    in_maps = []
    for r in range(RSH):
        for h in range(CSH):
            in_maps.append({"xa": xa_r[r], "sa": sa_h[h], "cb": cb_h[h], "xb": xb_r[r]})

    res = run_bass_kernel_spmd(nc, in_maps, list(range(NCORES)))

    final = np.empty((N, M), dtype=np.float32)
    for r in range(RSH):
        for h in range(CSH):
            piece = res.results[r * CSH + h]["out"]  # [16, 128, SC] f16
            final[r * SR : (r + 1) * SR, h * SC : (h + 1) * SC] = piece.reshape(
                SR, SC
            ).astype(np.float32)
    return final
